# revision 5
# baseline (speedup 1.0000x reference)
"""Trainium2 Bass kernel for the mock distributed MoE model.

Expert-parallel over 8 NeuronCores: core c owns expert c of both MoE layers
and vocab rows [4000c, 4000(c+1)) of the lm_head. Each core processes the
whole token batch densely (the reference computes every expert on every
token; only the combine is gated), the gated partial outputs are summed with
an fp32 AllReduce, and LayerNorm runs replicated on every core.

Numerics: fc1/fc2 matmuls run on the PE array in bf16 with a hi/lo split
(3 matmuls: ah@bh + ah@bl + al@bh -> ~4e-6 relative error) so routing
decisions match the fp32 reference. The layer-0 router runs in plain fp32 on
the exact gathered embeddings (the reference input contains a token whose
top-2 gap is 2.9e-8; only an fp32-exact router reproduces its selection);
the layer-1 router uses the same bf16x3 trick. LayerNorm statistics use
fp32 ones-matmuls; gelu runs on the ScalarE spline (~1e-7).
"""
import numpy as np
import ml_dtypes

import concourse.bass as bass
import concourse.tile as tile
from concourse import bacc, mybir
from concourse.bass import broadcast_tensor_aps, IndirectOffsetOnAxis
from concourse.bass_utils import run_bass_kernel_spmd

dt = mybir.dt
Alu = mybir.AluOpType
Act = mybir.ActivationFunctionType
AX = mybir.AxisListType

V, H, I, E, K, L = 32000, 1024, 4096, 8, 2, 2
B, S = 1, 2048
T = B * S
NC = 8
TC = 512                 # token chunk for fc1/fc2
NTC = T // TC            # 4
JPC = TC // 128          # 4 token tiles per chunk
NJ = T // 128            # 16 token tiles
KH = H // 128            # 8 H chunks
KI = I // 128            # 32 I chunks
IH = KI // 2             # 16 i-chunks per half
VS = V // NC             # 4000 vocab rows per core
VC = 250                 # vocab chunk
VCN = VS // VC           # 16

BF = ml_dtypes.bfloat16
_cache = {}


def _bf_pair(x):
    x = np.asarray(x, np.float32)
    hi = x.astype(BF)
    lo = (x - hi.astype(np.float32)).astype(BF)
    return hi, lo


def build():
    nc = bacc.Bacc("TRN2", target_bir_lowering=False, debug=False, num_devices=NC)

    d_ids = nc.dram_tensor("ids", [128, NJ], dt.int32, kind="ExternalInput").ap()
    d_emb = nc.dram_tensor("emb", [V, H], dt.float32, kind="ExternalInput").ap()
    d_ident = nc.dram_tensor("ident", [128, 128], dt.float32, kind="ExternalInput").ap()
    d_ones = nc.dram_tensor("ones_col", [128, 1], dt.float32, kind="ExternalInput").ap()
    d_onesrow = nc.dram_tensor("ones_row", [1, 128], dt.bfloat16, kind="ExternalInput").ap()
    d_rw32 = nc.dram_tensor("rw32", [H, E], dt.float32, kind="ExternalInput").ap()
    d_rwh = nc.dram_tensor("rwh", [H, E], dt.bfloat16, kind="ExternalInput").ap()
    d_rwl = nc.dram_tensor("rwl", [H, E], dt.bfloat16, kind="ExternalInput").ap()
    d_rb = nc.dram_tensor("rb", [E, L], dt.float32, kind="ExternalInput").ap()
    d_f1h = nc.dram_tensor("f1h", [L, KI, H, 128], dt.bfloat16, kind="ExternalInput").ap()
    d_f1l = nc.dram_tensor("f1l", [L, KI, H, 128], dt.bfloat16, kind="ExternalInput").ap()
    d_f2h = nc.dram_tensor("f2h", [L, KH, KI, 128, 128], dt.bfloat16, kind="ExternalInput").ap()
    d_f2l = nc.dram_tensor("f2l", [L, KH, KI, 128, 128], dt.bfloat16, kind="ExternalInput").ap()
    d_f1b = nc.dram_tensor("f1b", [128, L, KI], dt.float32, kind="ExternalInput").ap()
    d_f2b = nc.dram_tensor("f2b", [128, L, KH], dt.float32, kind="ExternalInput").ap()
    d_lng = nc.dram_tensor("lng", [128, L, KH], dt.float32, kind="ExternalInput").ap()
    d_lnb = nc.dram_tensor("lnb", [128, L, KH], dt.float32, kind="ExternalInput").ap()
    d_hw = nc.dram_tensor("hw", [VCN, H, VC], dt.bfloat16, kind="ExternalInput").ap()
    d_hbh = nc.dram_tensor("hbh", [1, VS], dt.bfloat16, kind="ExternalInput").ap()
    d_hbl = nc.dram_tensor("hbl", [1, VS], dt.bfloat16, kind="ExternalInput").ap()
    d_gsel = nc.dram_tensor("gsel", [128, E], dt.float32, kind="ExternalInput").ap()

    d_out = nc.dram_tensor("out", [T, VS], dt.float32, kind="ExternalOutput").ap()
    d_aux = nc.dram_tensor("aux", [1, 1], dt.float32, kind="ExternalOutput").ap()

    from contextlib import ExitStack
    with tile.TileContext(nc) as tc, ExitStack() as _es:
        _p = lambda **kw: _es.enter_context(tc.tile_pool(**kw))
        consts = _p(name="consts", bufs=1)
        hpool = _p(name="hpool", bufs=1)
        apool = _p(name="apool", bufs=1)
        opool = _p(name="opool", bufs=1)
        wstream = _p(name="wstream", bufs=4)
        hwstream = _p(name="hwstream", bufs=2)
        work = _p(name="work", bufs=2)
        abpool = _p(name="abpool", bufs=1)
        rpool = _p(name="rpool", bufs=1)
        dram = _p(name="dram", bufs=1, space="DRAM")
        ps_a = _p(name="ps_a", bufs=2, space="PSUM")
        ps_o = _p(name="ps_o", bufs=2, space="PSUM")
        ps_m = _p(name="ps_m", bufs=2, space="PSUM")
        ps_l = _p(name="ps_l", bufs=2, space="PSUM")
        if True:

            # ---------- constants ----------
            ids_t = consts.tile([128, NJ], dt.int32)
            nc.sync.dma_start(ids_t[:], d_ids)
            ident = consts.tile([128, 128], dt.float32)
            nc.sync.dma_start(ident[:], d_ident)
            ones_col = consts.tile([128, 1], dt.float32)
            nc.sync.dma_start(ones_col[:], d_ones)
            ones_row = consts.tile([1, 128], dt.bfloat16)
            nc.sync.dma_start(ones_row[:], d_onesrow)
            rw32 = consts.tile([128, KH, E], dt.float32)
            nc.sync.dma_start(rw32[:], d_rw32.rearrange("(k p) e -> p k e", p=128))
            rwh = consts.tile([128, KH, E], dt.bfloat16)
            nc.sync.dma_start(rwh[:], d_rwh.rearrange("(k p) e -> p k e", p=128))
            rwl = consts.tile([128, KH, E], dt.bfloat16)
            nc.sync.dma_start(rwl[:], d_rwl.rearrange("(k p) e -> p k e", p=128))
            rb = consts.tile([E, L], dt.float32)
            nc.sync.dma_start(rb[:], d_rb)
            f1b = consts.tile([128, L, KI], dt.float32)
            nc.sync.dma_start(f1b[:], d_f1b)
            f2b = consts.tile([128, L, KH], dt.float32)
            nc.sync.dma_start(f2b[:], d_f2b)
            lng = consts.tile([128, L, KH], dt.float32)
            nc.sync.dma_start(lng[:], d_lng)
            lnb = consts.tile([128, L, KH], dt.float32)
            nc.sync.dma_start(lnb[:], d_lnb)
            gsel = consts.tile([128, E], dt.float32)
            nc.sync.dma_start(gsel[:], d_gsel)
            aux_acc = consts.tile([1, 1], dt.float32)
            nc.vector.memset(aux_acc[:], 0.0)

            # persistent hidden state (bf16 hi/lo pair), in-place across layers
            hTh = hpool.tile([128, KH, T], dt.bfloat16, tag="hTh")
            hTl = hpool.tile([128, KH, T], dt.bfloat16, tag="hTl")
            lgT = rpool.tile([E, T], dt.float32, tag="lgT")
            g_row = rpool.tile([1, T], dt.float32, tag="g_row")

            ar_in = [dram.tile([H, T], dt.float32, tag=f"ar_in{l}", name=f"ar_in{l}") for l in range(L)]
            ar_out = [dram.tile([H, T], dt.float32, tag=f"ar_out{l}", name=f"ar_out{l}") for l in range(L)]

            # ---------- embedding gather + transpose + fp32 layer-0 router ----------
            for tcid in range(NTC):
                lg0 = ps_l.tile([E, TC], dt.float32, tag="lg0")
                for jj in range(JPC):
                    j = tcid * JPC + jj
                    h0 = work.tile([128, H], dt.float32, tag="h0g")
                    nc.gpsimd.indirect_dma_start(
                        out=h0[:], out_offset=None, in_=d_emb,
                        in_offset=IndirectOffsetOnAxis(ap=ids_t[:, j:j + 1], axis=0),
                    )
                    for k in range(KH):
                        tp = ps_m.tile([128, 128], dt.float32, tag="small")
                        nc.tensor.transpose(tp[:], h0[:, k * 128:(k + 1) * 128], ident[:])
                        h0T = work.tile([128, 128], dt.float32, tag="h0T")
                        nc.vector.tensor_copy(h0T[:], tp[:])
                        nc.vector.tensor_copy(hTh[:, k, j * 128:(j + 1) * 128], tp[:])
                        nc.vector.tensor_tensor(
                            hTl[:, k, j * 128:(j + 1) * 128], h0T[:],
                            hTh[:, k, j * 128:(j + 1) * 128], op=Alu.subtract)
                        nc.tensor.matmul(
                            lg0[:, jj * 128:(jj + 1) * 128], rw32[:, k, :], h0T[:],
                            start=(k == 0), stop=(k == KH - 1))
                nc.vector.tensor_scalar(
                    lgT[:, tcid * TC:(tcid + 1) * TC], lg0[:],
                    rb[:, 0:1], None, op0=Alu.add)

            for l in range(L):
                # ---------- router logits (layer >= 1, bf16x3) ----------
                if l > 0:
                    for tcid in range(NTC):
                        sl = slice(tcid * TC, (tcid + 1) * TC)
                        lgp = ps_l.tile([E, TC], dt.float32, tag="lg0")
                        step = 0
                        for k in range(KH):
                            for wa, ha in ((rwh, hTh), (rwl, hTh), (rwh, hTl)):
                                nc.tensor.matmul(
                                    lgp[:], wa[:, k, :], ha[:, k, sl],
                                    start=(step == 0), stop=(step == 3 * KH - 1))
                                step += 1
                        nc.vector.tensor_scalar(
                            lgT[:, sl], lgp[:], rb[:, l:l + 1], None, op0=Alu.add)

                # ---------- routing ----------
                lg_tok = rpool.tile([128, NJ, E], dt.float32, tag="lg_tok")
                for j in range(NJ):
                    tpl = ps_m.tile([128, 128], dt.float32, tag="small")
                    nc.tensor.transpose(
                        tpl[0:128, 0:E], lgT[:, j * 128:(j + 1) * 128], ident[0:E, 0:E])
                    nc.vector.tensor_copy(lg_tok[:, j, :], tpl[0:128, 0:E])

                m1 = rpool.tile([128, NJ], dt.float32, tag="m1")
                nc.vector.tensor_reduce(m1[:], lg_tok[:], axis=AX.X, op=Alu.max)
                m1v = m1[:].rearrange("p (s o) -> p s o", o=1)
                eqm = rpool.tile([128, NJ, E], dt.float32, tag="eqm")
                a_ap, b_ap = broadcast_tensor_aps(lg_tok[:], m1v)
                nc.vector.tensor_tensor(eqm[:], a_ap, b_ap, op=Alu.is_ge)
                tmp8 = rpool.tile([128, NJ, E], dt.float32, tag="tmp8")
                nc.vector.tensor_scalar(tmp8[:], eqm[:], 1e30, None, op0=Alu.mult)
                nc.vector.tensor_tensor(tmp8[:], lg_tok[:], tmp8[:], op=Alu.subtract)
                m2 = rpool.tile([128, NJ], dt.float32, tag="m2")
                nc.vector.tensor_reduce(m2[:], tmp8[:], axis=AX.X, op=Alu.max)
                m2v = m2[:].rearrange("p (s o) -> p s o", o=1)
                dwf = rpool.tile([128, NJ], dt.float32, tag="dwf")
                nc.vector.tensor_tensor(dwf[:], m2[:], m1[:], op=Alu.subtract)
                ez = rpool.tile([128, NJ], dt.float32, tag="ez")
                nc.scalar.activation(ez[:], dwf[:], Act.Exp)
                z1 = rpool.tile([128, NJ], dt.float32, tag="z1")
                nc.vector.tensor_scalar(z1[:], ez[:], 1.0, None, op0=Alu.add)
                rz = rpool.tile([128, NJ], dt.float32, tag="rz")
                nc.vector.reciprocal(rz[:], z1[:])
                mask = rpool.tile([128, NJ, E], dt.float32, tag="mask")
                a_ap, b_ap = broadcast_tensor_aps(lg_tok[:], m2v)
                nc.vector.tensor_tensor(mask[:], a_ap, b_ap, op=Alu.is_ge)
                e1 = rpool.tile([128, NJ, E], dt.float32, tag="e1")
                a_ap, b_ap = broadcast_tensor_aps(lg_tok[:], m1v)
                nc.vector.tensor_tensor(e1[:], a_ap, b_ap, op=Alu.subtract)
                e2 = rpool.tile([128, NJ, E], dt.float32, tag="e2")
                nc.scalar.activation(e2[:], e1[:], Act.Exp)
                nc.vector.tensor_tensor(e2[:], e2[:], mask[:], op=Alu.mult)
                rzv = rz[:].rearrange("p (s o) -> p s o", o=1)
                a_ap, b_ap = broadcast_tensor_aps(e2[:], rzv)
                nc.vector.tensor_tensor(e2[:], a_ap, b_ap, op=Alu.mult)

                cnt_ps = ps_m.tile([1, E], dt.float32, tag="small")
                for j in range(NJ):
                    nc.tensor.matmul(cnt_ps[0:1, 0:E], ones_col[:], mask[:, j, :],
                                     start=(j == 0), stop=(j == NJ - 1))
                wsum_ps = ps_m.tile([1, E], dt.float32, tag="small")
                for j in range(NJ):
                    nc.tensor.matmul(wsum_ps[0:1, 0:E], ones_col[:], e2[:, j, :],
                                     start=(j == 0), stop=(j == NJ - 1))
                cnt_s = rpool.tile([1, E], dt.float32, tag="cnt_s")
                nc.vector.tensor_copy(cnt_s[:], cnt_ps[0:1, 0:E])
                wsum_s = rpool.tile([1, E], dt.float32, tag="wsum_s")
                nc.vector.tensor_copy(wsum_s[:], wsum_ps[0:1, 0:E])
                cmax = rpool.tile([1, E], dt.float32, tag="cmax")
                nc.vector.tensor_scalar(cmax[:], cnt_s[:], 1.0, None, op0=Alu.max)
                rc = rpool.tile([1, E], dt.float32, tag="rc")
                nc.vector.reciprocal(rc[:], cmax[:])
                scal = rpool.tile([1, E], dt.float32, tag="scal")
                nc.vector.tensor_tensor(scal[:], wsum_s[:], rc[:], op=Alu.mult)

                dcnt = rpool.tile([1, E], dt.float32, tag="dcnt")
                nc.vector.tensor_scalar(dcnt[:], cnt_s[:], -float(T) / E, None, op0=Alu.add)
                nc.vector.tensor_tensor(dcnt[:], dcnt[:], dcnt[:], op=Alu.mult)
                auxl = rpool.tile([1, 1], dt.float32, tag="auxl")
                nc.vector.tensor_reduce(auxl[:], dcnt[:], axis=AX.X, op=Alu.add)
                nc.vector.tensor_scalar(auxl[:], auxl[:], 0.01 / E, None, op0=Alu.mult)
                nc.vector.tensor_tensor(aux_acc[:], aux_acc[:], auxl[:], op=Alu.add)

                # per-core gate g = mask[:, :, core] * scal[core] via one-hot gsel
                scal_b = rpool.tile([128, E], dt.float32, tag="scal_b")
                nc.gpsimd.partition_broadcast(scal_b[:], scal[:])
                gfull = rpool.tile([128, NJ, E], dt.float32, tag="gfull")
                sb_v = scal_b[:].rearrange("p (o e) -> p o e", o=1)
                a_ap, b_ap = broadcast_tensor_aps(mask[:], sb_v)
                nc.vector.tensor_tensor(gfull[:], a_ap, b_ap, op=Alu.mult)
                gs_v = gsel[:].rearrange("p (o e) -> p o e", o=1)
                a_ap, b_ap = broadcast_tensor_aps(gfull[:], gs_v)
                nc.vector.tensor_tensor(gfull[:], a_ap, b_ap, op=Alu.mult)
                g_tok = rpool.tile([128, NJ], dt.float32, tag="g_tok")
                nc.vector.tensor_reduce(g_tok[:], gfull[:], axis=AX.X, op=Alu.add)
                gt_ps = ps_m.tile([128, 128], dt.float32, tag="small")
                nc.tensor.transpose(gt_ps[0:NJ, 0:128], g_tok[:], ident[:])
                gT = rpool.tile([NJ, 128], dt.float32, tag="gT")
                nc.vector.tensor_copy(gT[:], gt_ps[0:NJ, 0:128])
                nc.sync.dma_start(g_row[0:1, :], gT[:])

                # ---------- fc1/fc2 over token chunks ----------
                for tcid in range(NTC):
                    sl = slice(tcid * TC, (tcid + 1) * TC)
                    g_b = work.tile([128, TC], dt.float32, tag="g_b")
                    nc.gpsimd.partition_broadcast(g_b[:], g_row[0:1, sl])
                    outacc = opool.tile([128, KH, TC], dt.float32, tag="outacc")
                    for ihalf in range(2):
                        aTh = apool.tile([128, IH, TC], dt.bfloat16, tag="aTh")
                        aTl = apool.tile([128, IH, TC], dt.bfloat16, tag="aTl")
                        for io in range(IH):
                            i = ihalf * IH + io
                            w1h = wstream.tile([128, KH, 128], dt.bfloat16, tag="w1h")
                            nc.sync.dma_start(
                                w1h[:], d_f1h[l, i].rearrange("(k p) m -> p k m", p=128))
                            w1l = wstream.tile([128, KH, 128], dt.bfloat16, tag="w1l")
                            nc.sync.dma_start(
                                w1l[:], d_f1l[l, i].rearrange("(k p) m -> p k m", p=128))
                            psa = ps_a.tile([128, TC], dt.float32, tag="psa")
                            step = 0
                            for k in range(KH):
                                for wa, ha in ((w1h, hTh), (w1l, hTh), (w1h, hTl)):
                                    nc.tensor.matmul(
                                        psa[:], wa[:, k, :], ha[:, k, sl],
                                        start=(step == 0), stop=(step == 3 * KH - 1))
                                    step += 1
                            gtmp = work.tile([128, TC], dt.float32, tag="gtmp")
                            nc.scalar.activation(gtmp[:], psa[:], Act.Gelu,
                                                 bias=f1b[:, l, i:i + 1])
                            nc.vector.tensor_copy(aTh[:, io, :], gtmp[:])
                            nc.vector.tensor_tensor(aTl[:, io, :], gtmp[:],
                                                    aTh[:, io, :], op=Alu.subtract)
                        for h in range(KH):
                            pso = ps_o.tile([128, TC], dt.float32, tag="pso")
                            step = 0
                            for iq in range(4):
                                w2h = wstream.tile([128, 4, 128], dt.bfloat16, tag="w2h")
                                w2l = wstream.tile([128, 4, 128], dt.bfloat16, tag="w2l")
                                i0 = ihalf * IH + iq * 4
                                nc.sync.dma_start(
                                    w2h[:], d_f2h[l, h, i0:i0 + 4].rearrange("i p q -> p i q"))
                                nc.sync.dma_start(
                                    w2l[:], d_f2l[l, h, i0:i0 + 4].rearrange("i p q -> p i q"))
                                for iw in range(4):
                                    io = iq * 4 + iw
                                    for wa, aa in ((w2h, aTh), (w2l, aTh), (w2h, aTl)):
                                        nc.tensor.matmul(
                                            pso[:], wa[:, iw, :], aa[:, io, :],
                                            start=(step == 0), stop=(step == 3 * IH - 1))
                                        step += 1
                            if ihalf == 0:
                                nc.vector.tensor_copy(outacc[:, h, :], pso[:])
                            else:
                                nc.vector.tensor_tensor(outacc[:, h, :], outacc[:, h, :],
                                                        pso[:], op=Alu.add)
                    for h in range(KH):
                        osc = work.tile([128, TC], dt.float32, tag="osc")
                        nc.vector.tensor_scalar(osc[:], outacc[:, h, :],
                                                f2b[:, l, h:h + 1], None, op0=Alu.add)
                        nc.vector.tensor_tensor(osc[:], osc[:], g_b[:], op=Alu.mult)
                        nc.sync.dma_start(ar_in[l][h * 128:(h + 1) * 128, sl], osc[:])

                # ---------- AllReduce over the 8 experts ----------
                nc.gpsimd.collective_compute(
                    "AllReduce", Alu.add,
                    replica_groups=[list(range(NC))],
                    ins=[ar_in[l].opt()], outs=[ar_out[l].opt()],
                )

                # ---------- LayerNorm -> next h (in place) ----------
                for tcid in range(NTC):
                    sl = slice(tcid * TC, (tcid + 1) * TC)
                    sums_ps = ps_m.tile([1, TC], dt.float32, tag="small")
                    sq_ps = ps_m.tile([1, TC], dt.float32, tag="small")
                    for k in range(KH):
                        oRk = work.tile([128, TC], dt.float32, tag="oRk")
                        nc.sync.dma_start(
                            oRk[:], ar_out[l][k * 128:(k + 1) * 128, sl])
                        nc.tensor.matmul(sums_ps[0:1, :], ones_col[:], oRk[:],
                                         start=(k == 0), stop=(k == KH - 1))
                        sqt = work.tile([128, TC], dt.float32, tag="gtmp")
                        nc.scalar.activation(sqt[:], oRk[:], Act.Square)
                        nc.tensor.matmul(sq_ps[0:1, :], ones_col[:], sqt[:],
                                         start=(k == 0), stop=(k == KH - 1))
                    mu = rpool.tile([1, TC], dt.float32, tag="mu")
                    nc.vector.tensor_scalar(mu[:], sums_ps[0:1, :], 1.0 / H, None, op0=Alu.mult)
                    ex2 = rpool.tile([1, TC], dt.float32, tag="ex2")
                    nc.vector.tensor_scalar(ex2[:], sq_ps[0:1, :], 1.0 / H, None, op0=Alu.mult)
                    mu2 = rpool.tile([1, TC], dt.float32, tag="mu2")
                    nc.vector.tensor_tensor(mu2[:], mu[:], mu[:], op=Alu.mult)
                    nc.vector.tensor_tensor(ex2[:], ex2[:], mu2[:], op=Alu.subtract)
                    nc.vector.tensor_scalar(ex2[:], ex2[:], 1e-5, None, op0=Alu.add)
                    sd = rpool.tile([1, TC], dt.float32, tag="mu2")
                    nc.scalar.activation(sd[:], ex2[:], Act.Sqrt)
                    rstd = rpool.tile([1, TC], dt.float32, tag="rstd")
                    nc.vector.reciprocal(rstd[:], sd[:])
                    nmu = rpool.tile([1, TC], dt.float32, tag="nmu")
                    nc.vector.tensor_tensor(nmu[:], mu[:], rstd[:], op=Alu.mult)
                    nc.vector.tensor_scalar(nmu[:], nmu[:], -1.0, None, op0=Alu.mult)
                    A_b = abpool.tile([128, TC], dt.float32, tag="A_b")
                    nc.gpsimd.partition_broadcast(A_b[:], rstd[:])
                    B_b = abpool.tile([128, TC], dt.float32, tag="B_b")
                    nc.gpsimd.partition_broadcast(B_b[:], nmu[:])
                    for k in range(KH):
                        oRk2 = work.tile([128, TC], dt.float32, tag="oRk")
                        nc.sync.dma_start(
                            oRk2[:], ar_out[l][k * 128:(k + 1) * 128, sl])
                        t1 = work.tile([128, TC], dt.float32, tag="osc")
                        nc.vector.tensor_tensor(t1[:], oRk2[:], A_b[:], op=Alu.mult)
                        nc.vector.tensor_tensor(t1[:], t1[:], B_b[:], op=Alu.add)
                        nc.vector.tensor_scalar(t1[:], t1[:], lng[:, l, k:k + 1],
                                                lnb[:, l, k:k + 1], op0=Alu.mult, op1=Alu.add)
                        nc.vector.tensor_copy(hTh[:, k, sl], t1[:])
                        nc.vector.tensor_tensor(hTl[:, k, sl], t1[:], hTh[:, k, sl],
                                                op=Alu.subtract)

            # ---------- lm_head (single bf16 + split bias) ----------
            for v in range(VCN):
                hw_t = hwstream.tile([128, KH, VC], dt.bfloat16, tag="hw_t")
                nc.sync.dma_start(hw_t[:], d_hw[v].rearrange("(k p) n -> p k n", p=128))
                hbh_t = hwstream.tile([1, VC], dt.bfloat16, tag="hbh_t")
                nc.sync.dma_start(hbh_t[:], d_hbh[0:1, v * VC:(v + 1) * VC])
                hbl_t = hwstream.tile([1, VC], dt.bfloat16, tag="hbl_t")
                nc.sync.dma_start(hbl_t[:], d_hbl[0:1, v * VC:(v + 1) * VC])
                for j in range(NJ):
                    psh = ps_a.tile([128, TC], dt.float32, tag="psa")
                    for k in range(KH):
                        nc.tensor.matmul(psh[:, 0:VC], hTh[:, k, j * 128:(j + 1) * 128],
                                         hw_t[:, k, :], start=(k == 0), stop=False)
                    nc.tensor.matmul(psh[:, 0:VC], ones_row[:], hbh_t[:],
                                     start=False, stop=False)
                    nc.tensor.matmul(psh[:, 0:VC], ones_row[:], hbl_t[:],
                                     start=False, stop=True)
                    outt = work.tile([128, VC], dt.float32, tag="osc")
                    nc.scalar.activation(outt[:], psh[:, 0:VC], Act.Copy)
                    nc.sync.dma_start(
                        d_out[j * 128:(j + 1) * 128, v * VC:(v + 1) * VC], outt[:])

            nc.sync.dma_start(d_aux, aux_acc[:])

    nc.compile()
    return nc


def prep_inputs(inputs):
    ids = np.asarray(inputs["input_ids"]).reshape(-1).astype(np.int32)
    ids_t = np.ascontiguousarray(ids.reshape(NJ, 128).T)
    emb = np.asarray(inputs["emb"], np.float32)
    router_w = np.asarray(inputs["router_w"], np.float32)
    router_b = np.asarray(inputs["router_b"], np.float32)
    fc1_w = np.asarray(inputs["fc1_w"], np.float32)
    fc1_b = np.asarray(inputs["fc1_b"], np.float32)
    fc2_w = np.asarray(inputs["fc2_w"], np.float32)
    fc2_b = np.asarray(inputs["fc2_b"], np.float32)
    ln_g = np.asarray(inputs["ln_g"], np.float32)
    ln_b = np.asarray(inputs["ln_b"], np.float32)
    head_w = np.asarray(inputs["head_w"], np.float32)
    head_b = np.asarray(inputs["head_b"], np.float32)

    ident = np.eye(128, dtype=np.float32)
    ones_col = np.ones((128, 1), np.float32)
    ones_row = np.ones((1, 128), BF)
    rw32 = np.ascontiguousarray(router_w[0].T)
    rwh, rwl = _bf_pair(router_w[1].T)
    rb = np.ascontiguousarray(router_b.T)
    lngp = np.ascontiguousarray(ln_g.reshape(L, KH, 128).transpose(2, 0, 1))
    lnbp = np.ascontiguousarray(ln_b.reshape(L, KH, 128).transpose(2, 0, 1))

    in_maps = []
    for c in range(NC):
        f1T = np.ascontiguousarray(fc1_w[:, c].transpose(0, 2, 1))      # [L, H, I]
        f1p = np.ascontiguousarray(
            f1T.reshape(L, H, KI, 128).transpose(0, 2, 1, 3))           # [L, KI, H, 128]
        f1h, f1l = _bf_pair(f1p)
        f2T = np.ascontiguousarray(fc2_w[:, c].transpose(0, 2, 1))      # [L, I, H]
        f2p = np.ascontiguousarray(
            f2T.reshape(L, KI, 128, KH, 128).transpose(0, 3, 1, 2, 4))  # [L,KH,KI,128,128]
        f2h, f2l = _bf_pair(f2p)
        f1bp = np.ascontiguousarray(
            fc1_b[:, c].reshape(L, KI, 128).transpose(2, 0, 1))
        f2bp = np.ascontiguousarray(
            fc2_b[:, c].reshape(L, KH, 128).transpose(2, 0, 1))
        hwT = np.ascontiguousarray(head_w[c * VS:(c + 1) * VS].T)       # [H, VS]
        hwp = np.ascontiguousarray(
            hwT.reshape(H, VCN, VC).transpose(1, 0, 2)).astype(BF)      # [VCN, H, VC]
        hbh_, hbl_ = _bf_pair(head_b[c * VS:(c + 1) * VS].reshape(1, VS))
        gsel = np.zeros((128, E), np.float32)
        gsel[:, c] = 1.0
        in_maps.append({
            "ids": ids_t, "emb": emb, "ident": ident, "ones_col": ones_col,
            "ones_row": ones_row, "rw32": rw32, "rwh": rwh, "rwl": rwl,
            "rb": rb, "f1h": f1h, "f1l": f1l, "f2h": f2h, "f2l": f2l,
            "f1b": f1bp, "f2b": f2bp, "lng": lngp, "lnb": lnbp,
            "hw": hwp, "hbh": hbh_, "hbl": hbl_, "gsel": gsel,
        })
    return in_maps


def kernel(**inputs):
    if "nc" not in _cache:
        _cache["nc"] = build()
    nc = _cache["nc"]
    in_maps = prep_inputs(inputs)
    r = run_bass_kernel_spmd(nc, in_maps, core_ids=list(range(NC)))
    logits = np.concatenate([r.results[c]["out"] for c in range(NC)], axis=1)
    aux = np.float32(r.results[0]["aux"][0, 0])
    return logits.reshape(B, S, V).astype(np.float32), aux
